# revision 14
# baseline (speedup 1.0000x reference)
"""Trainium2 Bass kernel for nn_Attention (seq2seq tanh-RNN with attention).

8-core SPMD strategy:
- Host pre-casts inputs to fp16 and pre-tiles every matrix into the exact
  transposed, contiguous layout the TensorEngine wants (contraction dim on
  partitions) -> all device DMAs are large contiguous copies at full rate.
- Input projections xw = x @ W_ih.T are sharded over n_class (contraction)
  across the 8 cores; partials combined with ONE fp32 AllReduce.
- The 512-step encoder scan and 256-step decoder scan run redundantly on
  every core entirely from SBUF (a per-step collective would cost ~10us).
  Per-step matvec: W^T-stationary fp16 tiles, h streams as a [128,1] column,
  producing h in partition-major layout that feeds the next step directly.
- Softmax normalization and the output projection are deferred out of the
  decoder loop: raw exp(scores) are stored; Z-sums, normalization, attns
  transposes and the [256,2048]@[2048,32000] projection run batched at the
  end.  out_W is sharded over n_class rows so each core writes a disjoint
  slice of `outs`.
- fp16 (not bf16) everywhere quantized: 8x finer mantissa at identical PE
  cost; all values here are O(1)-O(30) so range is safe, except exp(scores)
  which can reach ~1e19 and therefore stays bf16/fp32.
"""
import sys
sys.path.insert(0, '/opt/trn_rl_repo')

import numpy as np
import concourse.bass as bass
import concourse.bacc as bacc
import concourse.mybir as mybir
import concourse.tile as tile
from concourse.bass_utils import run_bass_kernel_spmd

F32 = mybir.dt.float32
F16 = mybir.dt.float16
BF16 = mybir.dt.bfloat16
AF = mybir.ActivationFunctionType
ALU = mybir.AluOpType

H = 1024
HT = H // 128  # 8 h-tiles


def build_kernel(n_cores=8, enc_len=512, dec_len=256, csh=4096, ocsh=4096,
                 debug=False):
    KT = csh // 128               # class k-tiles per core
    ST = enc_len // 128           # enc s-tiles
    DT2 = dec_len // 128          # dec t-tiles
    NCC = ocsh // 512             # out-proj class chunks per core
    TOT = enc_len + dec_len
    nc = bacc.Bacc(None, target_bir_lowering=False)

    # ---------------- DRAM I/O (host-pretiled, mostly fp16) ----------------
    xt_e = nc.dram_tensor("xt_e", [KT, 128, enc_len], F16, kind="ExternalInput")
    xt_d = nc.dram_tensor("xt_d", [KT, 128, dec_len], F16, kind="ExternalInput")
    wt_e = nc.dram_tensor("wt_e", [KT, 128, H], F16, kind="ExternalInput")
    wt_d = nc.dram_tensor("wt_d", [KT, 128, H], F16, kind="ExternalInput")
    wh_e = nc.dram_tensor("wh_e", [HT, 128, H], F16, kind="ExternalInput")
    wh_d = nc.dram_tensor("wh_d", [HT, 128, H], F16, kind="ExternalInput")
    awt = nc.dram_tensor("awt", [HT, 128, H], F16, kind="ExternalInput")
    owt = nc.dram_tensor("owt", [ocsh // 512, 128, 2 * HT, 512], F16,
                         kind="ExternalInput")
    b_enc = nc.dram_tensor("b_enc", [H], F32, kind="ExternalInput")  # b_ih+b_hh
    b_dec = nc.dram_tensor("b_dec", [H], F32, kind="ExternalInput")
    attn_b = nc.dram_tensor("attn_b", [H], F32, kind="ExternalInput")
    h0_d = nc.dram_tensor("h0", [H], F32, kind="ExternalInput")
    ob_row = nc.dram_tensor("ob_row", [ocsh], F16, kind="ExternalInput")

    outs_sl = nc.dram_tensor("outs_sl", [dec_len, ocsh], F32, kind="ExternalOutput")
    attns_o = nc.dram_tensor("attns_o", [dec_len, enc_len], F32, kind="ExternalOutput")
    if debug:
        dbg_hc = nc.dram_tensor("dbg_hc", [128, 2 * HT, dec_len], F16, kind="ExternalOutput")
        dbg_enc = nc.dram_tensor("dbg_enc", [128, HT, enc_len], F16, kind="ExternalOutput")
        dbg_xw = nc.dram_tensor("dbg_xw", [128, HT, enc_len], F16, kind="ExternalOutput")

    cc_in = nc.dram_tensor("cc_in", [H, TOT], F32)
    cc_out = nc.dram_tensor("cc_out", [H, TOT], F32, addr_space="Shared")

    with tile.TileContext(nc) as tc:
        with tc.tile_pool(name="persist", bufs=1) as pp:
            xw_e = pp.tile([128, HT, enc_len], F16, tag="xw_e")      # (p, j, t)
            xw_d = pp.tile([128, HT, dec_len], F16, tag="xw_d")
            enc_oT = pp.tile([128, HT, enc_len], F16, tag="enc_oT")  # (p, j, t)
            wh_e_sb = pp.tile([128, HT, H], F16, tag="wh_e_sb")      # (p, kk, hout)
            wh_d_sb = pp.tile([128, HT, H], F16, tag="wh_d_sb")
            at_sb = pp.tile([128, HT, enc_len], F16, tag="at_sb")    # A^T (p, kk, s)
            eos_sb = pp.tile([128, ST, H], F16, tag="eos_sb")        # enc_out s-major
            hc = pp.tile([128, 2 * HT, dec_len], F16, tag="hc")      # (p, jc, t)
            expw = pp.tile([128, ST, dec_len], F32, tag="expw")      # (p, ks, t)
            ctx_un = pp.tile([128, HT, dec_len], F32, tag="ctx_un")  # unnormalized
            b_e_pm = pp.tile([128, HT], F32, tag="b_e_pm")
            b_d_pm = pp.tile([128, HT], F32, tag="b_d_pm")
            ab_pm = pp.tile([128, HT], F32, tag="ab_pm")
            ident32 = pp.tile([128, 128], F32, tag="ident32")
            ident16 = pp.tile([128, 128], F16, tag="ident16")
            ones1 = pp.tile([128, 1], F32, tag="ones1")
            ones_r32 = pp.tile([1, 128], F32, tag="ones_r32")
            ones_r16 = pp.tile([1, 128], F16, tag="ones_r16")
            h0_16 = pp.tile([128, HT], F16, tag="h0_16")
            ob_sb = pp.tile([1, ocsh], F16, tag="ob_sb")

            # ---- constants ----
            nc.gpsimd.memset(ident32[:], 1.0)
            nc.gpsimd.affine_select(ident32[:], ident32[:], pattern=[[1, 128]],
                                    compare_op=ALU.is_equal, fill=0.0,
                                    base=0, channel_multiplier=-1)
            nc.vector.tensor_copy(ident16[:], ident32[:])
            nc.gpsimd.memset(ones1[:], 1.0)
            nc.gpsimd.memset(ones_r32[:], 1.0)
            nc.gpsimd.memset(ones_r16[:], 1.0)
            nc.sync.dma_start(b_e_pm[:], b_enc.rearrange("(j p) -> p j", p=128))
            nc.sync.dma_start(b_d_pm[:], b_dec.rearrange("(j p) -> p j", p=128))
            nc.sync.dma_start(ab_pm[:], attn_b.rearrange("(j p) -> p j", p=128))
            nc.sync.dma_start(ob_sb[:], ob_row.rearrange("(o c) -> o c", o=1))
            h0_32 = pp.tile([128, HT], F32, tag="h0_32")
            nc.sync.dma_start(h0_32[:], h0_d.rearrange("(j p) -> p j", p=128))
            nc.vector.tensor_copy(h0_16[:], h0_32[:])
            for kk in range(HT):
                nc.sync.dma_start(wh_e_sb[:, kk, :], wh_e[kk])
                nc.sync.dma_start(wh_d_sb[:, kk, :], wh_d[kk])

            # ---------------- Phase 1: input projections ----------------
            for which, xsrc, wsrc, slen, coff in (
                    (0, xt_e, wt_e, enc_len, 0),
                    (1, xt_d, wt_d, dec_len, enc_len)):
                with (tc.tile_pool(name=f"proj{which}", bufs=3) as prp,
                      tc.tile_pool(name=f"projp{which}", bufs=1, space="PSUM") as ppp):
                    psums = [ppp.tile([128, slen], F32, tag=f"ps{m}", name=f"ps{m}")
                             for m in range(HT)]
                    for k in range(KT):
                        xt = prp.tile([128, slen], F16, tag="xt")
                        nc.sync.dma_start(xt[:], xsrc[k])
                        wt = prp.tile([128, H], F16, tag="wt")
                        nc.sync.dma_start(wt[:], wsrc[k])
                        for m in range(HT):
                            nc.tensor.matmul(psums[m][:], wt[:, m * 128:(m + 1) * 128],
                                             xt[:], start=(k == 0), stop=(k == KT - 1))
                    for m in range(HT):
                        sb = prp.tile([128, slen], F32, tag="sb")
                        nc.vector.tensor_copy(sb[:], psums[m][:])
                        nc.sync.dma_start(
                            cc_in[m * 128:(m + 1) * 128, coff:coff + slen], sb[:])

            # ---------------- AllReduce of partial xw ----------------
            if n_cores > 1:
                nc.gpsimd.collective_compute(
                    "AllReduce", ALU.add, ins=[cc_in[:]], outs=[cc_out[:]],
                    replica_groups=[list(range(n_cores))])
                xw_src = cc_out
            else:
                xw_src = cc_in
            for j in range(HT):
                nc.gpsimd.dma_start(xw_e[:, j, :],
                                    xw_src[j * 128:(j + 1) * 128, 0:enc_len])
                nc.gpsimd.dma_start(xw_d[:, j, :],
                                    xw_src[j * 128:(j + 1) * 128, enc_len:TOT])
            for j in range(HT):
                nc.vector.tensor_scalar_add(xw_e[:, j, :], xw_e[:, j, :],
                                            b_e_pm[:, j:j + 1])
                nc.vector.tensor_scalar_add(xw_d[:, j, :], xw_d[:, j, :],
                                            b_d_pm[:, j:j + 1])

            # ---------------- Phase 2: encoder scan ----------------
            with (tc.tile_pool(name="scan", bufs=2) as sp,
                  tc.tile_pool(name="scanp", bufs=1, space="PSUM") as spp):
                psum_h = spp.tile([128, HT], F32, tag="psh")
                pre = sp.tile([128, HT], F32, tag="pre")
                for i in range(enc_len):
                    for m in range(HT):
                        for kk in range(HT):
                            rhs = (h0_16[:, kk:kk + 1] if i == 0
                                   else enc_oT[:, kk, i - 1:i])
                            nc.tensor.matmul(psum_h[:, m:m + 1],
                                             wh_e_sb[:, kk, m * 128:(m + 1) * 128],
                                             rhs, start=(kk == 0), stop=(kk == HT - 1))
                    nc.vector.scalar_tensor_tensor(pre[:], psum_h[:], 0.0,
                                                   xw_e[:, :, i], op0=ALU.add,
                                                   op1=ALU.add)
                    nc.scalar.activation(enc_oT[:, :, i], pre[:], AF.Tanh)

            # ---------------- Phase 3: A^T and enc_out s-major ----------------
            with (tc.tile_pool(name="aph", bufs=1) as ap_,
                  tc.tile_pool(name="aphp", bufs=1, space="PSUM") as app):
                awt_sb = ap_.tile([128, HT, H], F16, tag="awt_sb")
                for kk in range(HT):
                    nc.sync.dma_start(awt_sb[:, kk, :], awt[kk])
                psum_a = app.tile([128, enc_len], F32, tag="psa")
                for m in range(HT):
                    for kk in range(HT):
                        nc.tensor.matmul(psum_a[:],
                                         awt_sb[:, kk, m * 128:(m + 1) * 128],
                                         enc_oT[:, kk, :],
                                         start=(kk == 0), stop=(kk == HT - 1))
                    nc.vector.tensor_scalar_add(at_sb[:, m, :], psum_a[:],
                                                ab_pm[:, m:m + 1])
                psum_t = app.tile([128, 128], F16, tag="pst")
                for ks in range(ST):
                    for j in range(HT):
                        nc.tensor.transpose(psum_t[:],
                                            enc_oT[:, j, ks * 128:(ks + 1) * 128],
                                            ident16[:])
                        nc.vector.tensor_copy(eos_sb[:, ks, j * 128:(j + 1) * 128],
                                              psum_t[:])

            # ---------------- Phase 4: decoder scan ----------------
            KS = ST
            with (tc.tile_pool(name="dscan", bufs=2) as dp,
                  tc.tile_pool(name="dscanp", bufs=1, space="PSUM") as dpp):
                psum_h2 = dpp.tile([128, HT], F32, tag="psh2")
                psum_s = dpp.tile([128, KS], F32, tag="pss")
                psum_c = dpp.tile([128, HT], F32, tag="psc")
                pre2 = dp.tile([128, HT], F32, tag="pre2")
                ew_bf = dp.tile([128, KS], BF16, tag="ew_bf")
                for i in range(dec_len):
                    for m in range(HT):
                        for kk in range(HT):
                            rhs = (enc_oT[:, kk, enc_len - 1:enc_len] if i == 0
                                   else hc[:, kk, i - 1:i])
                            nc.tensor.matmul(psum_h2[:, m:m + 1],
                                             wh_d_sb[:, kk, m * 128:(m + 1) * 128],
                                             rhs, start=(kk == 0), stop=(kk == HT - 1))
                    nc.vector.scalar_tensor_tensor(pre2[:], psum_h2[:], 0.0,
                                                   xw_d[:, :, i], op0=ALU.add,
                                                   op1=ALU.add)
                    nc.scalar.activation(hc[:, 0:HT, i], pre2[:], AF.Tanh)
                    for m4 in range(KS):
                        for kk in range(HT):
                            nc.tensor.matmul(psum_s[:, m4:m4 + 1],
                                             at_sb[:, kk, m4 * 128:(m4 + 1) * 128],
                                             hc[:, kk, i:i + 1],
                                             start=(kk == 0), stop=(kk == HT - 1))
                    nc.scalar.activation(expw[:, :, i], psum_s[:], AF.Exp)
                    nc.vector.tensor_copy(ew_bf[:], expw[:, :, i])
                    for m in range(HT):
                        for ks in range(KS):
                            nc.tensor.matmul(psum_c[:, m:m + 1],
                                             eos_sb[:, ks, m * 128:(m + 1) * 128],
                                             ew_bf[:, ks:ks + 1],
                                             start=(ks == 0), stop=(ks == KS - 1))
                    nc.vector.tensor_copy(ctx_un[:, :, i], psum_c[:])

            # ---------------- Phase 5: attns + context normalization --------
            with (tc.tile_pool(name="att", bufs=1) as atp,
                  tc.tile_pool(name="attp", bufs=1, space="PSUM") as atpp):
                psum_zt = atpp.tile([128, DT2], F32, tag="pszt")
                for tcc in range(DT2):
                    for ks in range(KS):
                        nc.tensor.matmul(psum_zt[:, tcc:tcc + 1],
                                         expw[:, ks, tcc * 128:(tcc + 1) * 128],
                                         ones1[:], start=(ks == 0),
                                         stop=(ks == KS - 1))
                rzt = atp.tile([128, DT2], F32, tag="rzt")
                nc.vector.reciprocal(rzt[:], psum_zt[:])
                attn_tm = atp.tile([128, DT2, enc_len], F32, tag="attn_tm")
                psum_tr = atpp.tile([128, 128], F32, tag="pstr")
                for j in range(KS):
                    for tcc in range(DT2):
                        nc.tensor.transpose(psum_tr[:],
                                            expw[:, j, tcc * 128:(tcc + 1) * 128],
                                            ident32[:])
                        nc.vector.tensor_scalar_mul(
                            attn_tm[:, tcc, j * 128:(j + 1) * 128],
                            psum_tr[:], rzt[:, tcc:tcc + 1])
                nc.sync.dma_start(
                    attns_o.rearrange("(tc p) s -> p tc s", p=128), attn_tm[:])
                # row-layout 1/Z -> broadcast across partitions -> scale context
                zrow = atp.tile([1, dec_len], F32, tag="zrow")
                CH = max(1, (dec_len * KS) // 512)
                KCH = max(1, KS // CH)
                expw_f = expw.rearrange("p a t -> p (a t)")
                for c in range(CH):
                    w512 = (dec_len * KS) // CH
                    psum_zr = atpp.tile([1, w512], F32, tag="pszr")
                    nc.tensor.matmul(psum_zr[:], ones1[:],
                                     expw_f[:, c * w512:(c + 1) * w512],
                                     start=True, stop=True)
                    zr_v = psum_zr.rearrange("o (ks t) -> o ks t", t=dec_len)
                    for ksl in range(KCH):
                        if c == 0 and ksl == 0:
                            nc.vector.tensor_copy(zrow[:], zr_v[:, 0, :])
                        else:
                            nc.vector.scalar_tensor_tensor(
                                zrow[:], zrow[:], 0.0, zr_v[:, ksl, :],
                                op0=ALU.add, op1=ALU.add)
                rzrow = atp.tile([1, dec_len], F32, tag="rzrow")
                nc.vector.reciprocal(rzrow[:], zrow[:])
                psum_bc = atpp.tile([128, dec_len], F32, tag="psbc")
                nc.tensor.matmul(psum_bc[:], ones_r32[:], rzrow[:],
                                 start=True, stop=True)
                for j in range(HT):
                    nc.vector.scalar_tensor_tensor(hc[:, HT + j, :], ctx_un[:, j, :],
                                                   0.0, psum_bc[:], op0=ALU.add,
                                                   op1=ALU.mult)

            if debug:
                nc.sync.dma_start(dbg_hc[:], hc[:])
                nc.sync.dma_start(dbg_enc[:], enc_oT[:])
                nc.sync.dma_start(dbg_xw[:], xw_e[:])

            # ---------------- Phase 6: output projection ----------------
            with (tc.tile_pool(name="oproj", bufs=2) as op_,
                  tc.tile_pool(name="oprojp", bufs=2, space="PSUM") as opp):
                for ncc in range(ocsh // 512):
                    owt_sb = op_.tile([128, 2 * HT, 512], F16, tag="owt_sb")
                    nc.sync.dma_start(owt_sb[:], owt[ncc])
                    for m in range(DT2):
                        psum_o = opp.tile([128, 512], F32, tag="pso")
                        for j in range(2 * HT):
                            nc.tensor.matmul(psum_o[:],
                                             hc[:, j, m * 128:(m + 1) * 128],
                                             owt_sb[:, j, :],
                                             start=(j == 0), stop=False)
                        nc.tensor.matmul(psum_o[:], ones_r16[:],
                                         ob_sb[:, ncc * 512:(ncc + 1) * 512],
                                         start=False, stop=True)
                        osb = op_.tile([128, 512], F32, tag="osb")
                        nc.vector.tensor_copy(osb[:], psum_o[:])
                        nc.sync.dma_start(
                            outs_sl[m * 128:(m + 1) * 128,
                                    ncc * 512:(ncc + 1) * 512], osb[:])

    nc.compile()
    return nc


# ---------------------------------------------------------------------------
_CACHED = {}


def _get_nc(cfg):
    if cfg not in _CACHED:
        _CACHED[cfg] = build_kernel(*cfg)
    return _CACHED[cfg]


def kernel(enc_input, hidden, dec_input,
           enc_W_ih, enc_W_hh, enc_b_ih, enc_b_hh,
           dec_W_ih, dec_W_hh, dec_b_ih, dec_b_hh,
           attn_W, attn_b, out_W, out_b):
    N_CORES = 8
    ENC_LEN, DEC_LEN, NCLS = 512, 256, 32000
    CSH = 4096           # padded class shard (8*4096 >= 32000)
    OCSH = 4096          # padded out_W row shard (trimmed to 4000 on host)

    nc = _get_nc((N_CORES, ENC_LEN, DEC_LEN, CSH, OCSH, False))

    f16 = np.float16
    PAD = N_CORES * CSH  # 32768
    KT = CSH // 128

    def padT(a):  # [S, NCLS] -> fp16 [PAD, S] transposed zero-padded
        out = np.zeros((PAD, a.shape[0]), f16)
        out[:NCLS] = np.ascontiguousarray(np.asarray(a, np.float32).T).astype(f16)
        return out

    x_encT = padT(enc_input[0])
    x_decT = padT(dec_input[0])
    wT_e = padT(enc_W_ih)
    wT_d = padT(dec_W_ih)
    wh_e3 = np.ascontiguousarray(np.asarray(enc_W_hh, np.float32).T).astype(f16).reshape(HT, 128, H)
    wh_d3 = np.ascontiguousarray(np.asarray(dec_W_hh, np.float32).T).astype(f16).reshape(HT, 128, H)
    awt3 = np.ascontiguousarray(np.asarray(attn_W, np.float32).T).astype(f16).reshape(HT, 128, H)
    b_enc = np.asarray(enc_b_ih + enc_b_hh, np.float32)
    b_dec = np.asarray(dec_b_ih + dec_b_hh, np.float32)
    attn_b32 = np.ascontiguousarray(attn_b, np.float32)
    h0 = np.ascontiguousarray(hidden[0, 0], np.float32)
    out_W16 = np.asarray(out_W, np.float32).astype(f16)
    out_b16 = np.asarray(out_b, np.float32).astype(f16)

    in_maps = []
    for c in range(N_CORES):
        lo = c * CSH
        o_lo = c * 4000
        ow = np.zeros((OCSH, 2 * H), f16)
        ow[:4000] = out_W16[o_lo:o_lo + 4000]
        owT = np.ascontiguousarray(ow.T)  # [2048, OCSH]
        owt_arr = np.ascontiguousarray(
            owT.reshape(2 * HT, 128, OCSH // 512, 512).transpose(2, 1, 0, 3))
        ob = np.zeros(OCSH, f16)
        ob[:4000] = out_b16[o_lo:o_lo + 4000]
        in_maps.append({
            "xt_e": x_encT[lo:lo + CSH].reshape(KT, 128, ENC_LEN),
            "xt_d": x_decT[lo:lo + CSH].reshape(KT, 128, DEC_LEN),
            "wt_e": wT_e[lo:lo + CSH].reshape(KT, 128, H),
            "wt_d": wT_d[lo:lo + CSH].reshape(KT, 128, H),
            "wh_e": wh_e3, "wh_d": wh_d3, "awt": awt3, "owt": owt_arr,
            "b_enc": b_enc, "b_dec": b_dec, "attn_b": attn_b32, "h0": h0,
            "ob_row": ob,
        })

    res = run_bass_kernel_spmd(nc, in_maps, list(range(N_CORES)))
    outs = np.concatenate(
        [res.results[c]["outs_sl"][:, :4000] for c in range(N_CORES)], axis=1)
    attns = res.results[0]["attns_o"]
    return outs, attns
